# revision 13
# baseline (speedup 1.0000x reference)
"""BD3LM block-diffusion decoder layer on 8 trn2 NeuronCores.

Sharding: core = 2*b + g  (b = batch 0..3, g = head-group 0..1, 8 heads each).
Each core: QKV projections for its batch/head-group, sparse BD3LM attention
(only ~80 of 256 score tiles per head), O-projection against its Wo row-slice.
Host: sums the two group partials per batch and adds the (bv @ Wo + bo)
correction (softmax rows sum to 1, so the v-bias contributes exactly bv @ Wo).

Layouts on device (per core):
  qT/kT  [d_head_group=512, T=2048]  stored [128, 4, 2048]  (d on partitions)
  v      [T, 512] stored [128, 16, 8*65] with a per-head ones column -> the
         ctx matmul accumulates softmax denominators for free (row 64).
  scores computed transposed [k_tile=128, q_span] so softmax reduction is a
         PE matmul instead of a partition reduction; exp on ACT without
         max-subtraction (scores are ~N(0,1), bias-free overflow impossible);
         only 3 distinct 128x128 binary mask tiles (strict/incl/diag).
"""

import numpy as np

import concourse.bass as bass
import concourse.mybir as mybir
import concourse.tile as tile
from concourse import bacc
from concourse.bass_utils import run_bass_kernel_spmd

F32 = mybir.dt.float32
F32R = mybir.dt.float32r
Act = mybir.ActivationFunctionType

B, T, D = 4, 2048, 1024
H, HD = 16, 64
L = T // 2           # 1024, length of each of [xt | x0]
BS = 4               # block size
G = 2                # head groups (cores per batch)
DG = D // G          # 512 channels per group
HG = H // G          # 8 heads per core
P = 128
NT = L // P          # 8 key/query tiles per half
SLAB = 256           # projection t-slab width

# matmul dtype per family: float32 (exact, 4 cyc/row) or float32r (~1.5e-4, 1 cyc/row)
PROJ_DT = F32
ATTN_DT = F32
OPROJ_DT = F32

DT4_C = 4  # DG // P

_CACHE = {}


def _chunks512(a0, a1):
    """Split [a0, a1) at multiples of 512 (PSUM bank boundaries)."""
    out = []
    while a0 < a1:
        b1 = min(a1, (a0 // 512 + 1) * 512)
        out.append((a0, b1))
        a0 = b1
    return out


def _mm(ap, dt):
    return ap.bitcast(dt) if dt != F32 else ap


DBG = False


def _build():
    import concourse.tile_utils as tile_utils

    tile_utils.max_sbuf_usage = 204 * 1024  # trn2 has 208KB/partition usable

    nc = bacc.Bacc("TRN2", target_bir_lowering=False, debug=False, num_devices=8)
    dbg = {}
    if DBG:
        for nm, shp in (
            ("dbg_qT", [P, DT4_C, T]),
            ("dbg_kT", [P, DT4_C, T]),
            ("dbg_v", [P, T // P, HG * (HD + 1)]),
            ("dbg_ctxT", [P, DT4_C, T]),
            ("dbg_nd", [16, HD + 1, L]),
            ("dbg_at", [P, L]),
        ):
            dbg[nm] = nc.dram_tensor(nm, shp, F32, kind="ExternalOutput").ap()

    xT = nc.dram_tensor("xT", [D, T], F32, kind="ExternalInput").ap()
    wq = nc.dram_tensor("wq", [D, DG], F32, kind="ExternalInput").ap()
    wk = nc.dram_tensor("wk", [D, DG], F32, kind="ExternalInput").ap()
    wv = nc.dram_tensor("wv", [D, DG], F32, kind="ExternalInput").ap()
    wo = nc.dram_tensor("wo", [DG, D], F32, kind="ExternalInput").ap()
    bqs = nc.dram_tensor("bqs", [DG], F32, kind="ExternalInput").ap()
    bks = nc.dram_tensor("bks", [DG], F32, kind="ExternalInput").ap()
    msk = nc.dram_tensor("msk", [3, P, P], F32, kind="ExternalInput").ap()
    out = nc.dram_tensor("out", [T, D], F32, kind="ExternalOutput").ap()

    xT_v = xT.rearrange("(kc p) t -> p kc t", p=P)      # [128, 8, 2048]
    wq_v = wq.rearrange("(kc p) m -> p kc m", p=P)      # [128, 8, 512]
    wk_v = wk.rearrange("(kc p) m -> p kc m", p=P)
    wv_v = wv.rearrange("(kc p) m -> p kc m", p=P)
    wo_v = wo.rearrange("(cc p) n -> p cc n", p=P)      # [128, 4, 1024]
    bq_v = bqs.rearrange("(c p) -> p c", p=P)           # [128, 4]
    bk_v = bks.rearrange("(c p) -> p c", p=P)

    KC = D // P   # 8 contraction chunks
    DT4 = DG // P  # 4 output-partition tiles for qT/kT

    with tile.TileContext(nc) as tc:
        with tc.tile_pool(name="persist", bufs=1) as pers:
            qT_sb = pers.tile([P, DT4, T], F32)
            kT_sb = pers.tile([P, DT4, T], F32)
            v_sb = pers.tile([P, T // P, HG * (HD + 1)], F32)   # [128, 16, 520]
            ctxT_sb = pers.tile([P, DT4, T], F32)
            bq_sb = pers.tile([P, DT4], F32)
            bk_sb = pers.tile([P, DT4], F32)
            nc.sync.dma_start(bq_sb, bq_v)
            nc.sync.dma_start(bk_sb, bk_v)
            # ones columns for the softmax denominators
            ones_v = v_sb.rearrange("p t (h c) -> p t h c", c=HD + 1)[:, :, :, HD : HD + 1]
            nc.vector.memset(ones_v, 1.0)

            # ---------------- Phase A: QKV projections ----------------
            with (
                tc.tile_pool(name="wpool", bufs=1) as wpool,
                tc.tile_pool(name="xpool", bufs=2) as xpool,
                tc.tile_pool(name="ppsum", bufs=4, space="PSUM") as ppsum,
            ):
                wq_sb = wpool.tile([P, KC, DG], F32)
                wk_sb = wpool.tile([P, KC, DG], F32)
                wv_sb = wpool.tile([P, KC, DG], F32)
                nc.sync.dma_start(wq_sb, wq_v)
                nc.sync.dma_start(wk_sb, wk_v)
                nc.sync.dma_start(wv_sb, wv_v)

                for s in range(T // SLAB):
                    x_sb = xpool.tile([P, KC, SLAB], F32, tag="x", name=f"x{s}")
                    nc.sync.dma_start(x_sb, xT_v[:, :, SLAB * s : SLAB * (s + 1)])
                    for w_sb, b_sb, dst, scale in (
                        (wq_sb, bq_sb, qT_sb, HD ** -0.5),
                        (wk_sb, bk_sb, kT_sb, 1.0),
                    ):
                        for d4 in range(DT4):
                            ps = ppsum.tile([P, SLAB], F32, tag="pp", name=f"pp{s}_{d4}")
                            for kc in range(KC):
                                nc.tensor.matmul(
                                    ps,
                                    _mm(w_sb[:, kc, P * d4 : P * (d4 + 1)], PROJ_DT),
                                    _mm(x_sb[:, kc, :], PROJ_DT),
                                    start=(kc == 0),
                                    stop=(kc == KC - 1),
                                )
                            nc.scalar.activation(
                                dst[:, d4, SLAB * s : SLAB * (s + 1)],
                                ps,
                                Act.Identity,
                                bias=b_sb[:, d4 : d4 + 1],
                                scale=scale,
                            )
                    for t2 in range(SLAB // P):
                        tt = (SLAB // P) * s + t2
                        ps = ppsum.tile([P, DG], F32, tag="ppv", name=f"ppv{tt}")
                        for kc in range(KC):
                            nc.tensor.matmul(
                                ps,
                                _mm(x_sb[:, kc, P * t2 : P * (t2 + 1)], PROJ_DT),
                                _mm(wv_sb[:, kc, :], PROJ_DT),
                                start=(kc == 0),
                                stop=(kc == KC - 1),
                            )
                        nc.vector.tensor_copy(
                            v_sb[:, tt].rearrange("p (h c) -> p h c", c=HD + 1)[:, :, :HD],
                            ps.rearrange("p (h c) -> p h c", c=HD),
                        )

            # ---------------- Phase B: sparse attention ----------------
            with (
                tc.tile_pool(name="apool", bufs=1) as apool,
                tc.tile_pool(name="atpool", bufs=6) as atpool,
                tc.tile_pool(name="tmppool", bufs=2) as tmppool,
                tc.tile_pool(name="spsum", bufs=2, space="PSUM") as spsum,
                tc.tile_pool(name="cpsum", bufs=2, space="PSUM") as cpsum,
            ):
                wo_sb = apool.tile([P, DT4, D], F32)
                nc.sync.dma_start(wo_sb, wo_v)
                m_strict = apool.tile([P, P], F32)
                m_incl = apool.tile([P, P], F32)
                m_diag = apool.tile([P, P], F32)
                nc.sync.dma_start(m_strict, msk[0])
                nc.sync.dma_start(m_incl, msk[1])
                nc.sync.dma_start(m_diag, msk[2])
                ones_t = apool.tile([P, HD], F32)  # row 64 used as K=1 bcast lhsT
                nc.vector.memset(ones_t, 1.0)

                for h in range(HG):
                    c, p0 = h // 2, HD * (h % 2)
                    qh = qT_sb[p0 : p0 + HD, c, :]   # [64, 2048]
                    kh = kT_sb[p0 : p0 + HD, c, :]
                    ctx_ps = [
                        cpsum.tile([HD + 1, L], F32, tag="ctx", name=f"ctx{h}_{half}")
                        for half in range(2)
                    ]
                    # last writer per (half, bank) sets stop=True:
                    # xt half: diag matmuls finish each bank; x0 half: the j-loop does.
                    for j in range(NT):
                        kv = kh[:, L + P * j : L + P * (j + 1)]           # [64, 128]
                        vj = v_sb[:, NT + j, (HD + 1) * h : (HD + 1) * (h + 1)]  # [128, 65]
                        for half in range(2):
                            mask = m_strict if half == 0 else m_incl
                            for a0, a1 in _chunks512(P * j, L):
                                n = a1 - a0
                                sc = spsum.tile(
                                    [P, 512], F32, tag="sc", name=f"sc{h}_{j}_{half}_{a0}"
                                )[:, :n]
                                nc.tensor.matmul(
                                    sc,
                                    _mm(kv, ATTN_DT),
                                    _mm(qh[:, L * half + a0 : L * half + a1], ATTN_DT),
                                    start=True,
                                    stop=True,
                                )
                                at = atpool.tile(
                                    [P, 512], F32, tag="at", name=f"at{h}_{j}_{half}_{a0}"
                                )[:, :n]
                                nc.scalar.activation(at, sc, Act.Exp)
                                if a0 == P * j:
                                    nc.vector.tensor_mul(at[:, :P], at[:, :P], mask)
                                if DBG and h == 0 and j == 0 and half == 1:
                                    nc.sync.dma_start(dbg["dbg_at"][:, a0:a1], at)
                                # x0 half: stop on the last j touching this bank
                                last = half == 1 and (
                                    (a1 <= 512 and j == 3) or (a0 >= 512 and j == NT - 1)
                                )
                                nc.tensor.matmul(
                                    ctx_ps[half][:, a0:a1],
                                    _mm(vj, ATTN_DT),
                                    _mm(at, ATTN_DT),
                                    start=(j == 0),
                                    stop=last,
                                )
                    # xt-xt block-diagonal tiles
                    for i in range(NT):
                        scd = spsum.tile([P, 512], F32, tag="sc", name=f"scd{h}_{i}")[:, :P]
                        nc.tensor.matmul(
                            scd,
                            _mm(kh[:, P * i : P * (i + 1)], ATTN_DT),
                            _mm(qh[:, P * i : P * (i + 1)], ATTN_DT),
                            start=True,
                            stop=True,
                        )
                        atd = atpool.tile([P, 512], F32, tag="at", name=f"atd{h}_{i}")[:, :P]
                        nc.scalar.activation(atd, scd, Act.Exp)
                        nc.vector.tensor_mul(atd, atd, m_diag)
                        nc.tensor.matmul(
                            ctx_ps[0][:, P * i : P * (i + 1)],
                            _mm(v_sb[:, i, (HD + 1) * h : (HD + 1) * (h + 1)], ATTN_DT),
                            _mm(atd, ATTN_DT),
                            start=False,
                            stop=(i == 3 or i == NT - 1),
                        )
                    if DBG:
                        for half in range(2):
                            ndc = tmppool.tile(
                                [HD + 1, L], F32, tag="ndc", name=f"ndc{h}_{half}"
                            )
                            nc.scalar.activation(ndc, ctx_ps[half], Act.Copy)
                            nc.sync.dma_start(dbg["dbg_nd"][2 * h + half], ndc)
                    # normalize: ctxT = ctx[:64] * (1 / denom), denom = row 64
                    for half in range(2):
                        recip = tmppool.tile([P, L], F32, tag="recip", name=f"rc{h}_{half}")
                        nc.vector.reciprocal(
                            recip[HD : HD + 1, :], ctx_ps[half][HD : HD + 1, :]
                        )
                        rb = tmppool.tile([HD, L], F32, tag="rb", name=f"rb{h}_{half}")
                        # PE broadcast: ones[1,64].T @ recip[1,n] -> [64, n]
                        for c0 in range(0, L, 512):
                            bc = spsum.tile(
                                [P, 512], F32, tag="sc", name=f"bc{h}_{half}_{c0}"
                            )[:HD, :]
                            nc.tensor.matmul(
                                bc,
                                ones_t[HD : HD + 1, :],
                                recip[HD : HD + 1, c0 : c0 + 512],
                                start=True,
                                stop=True,
                            )
                            nc.scalar.activation(rb[:, c0 : c0 + 512], bc, Act.Copy)
                        if h % 2 == 0:
                            nc.vector.tensor_mul(
                                ctxT_sb[:HD, c, L * half : L * (half + 1)],
                                ctx_ps[half][:HD, :],
                                rb,
                            )
                        else:
                            cs = tmppool.tile([HD, L], F32, tag="cs", name=f"cs{h}_{half}")
                            nc.vector.tensor_mul(cs, ctx_ps[half][:HD, :], rb)
                            nc.sync.dma_start(
                                ctxT_sb[HD : 2 * HD, c, L * half : L * (half + 1)], cs
                            )

                if DBG:
                    nc.sync.dma_start(dbg["dbg_qT"], qT_sb)
                    nc.sync.dma_start(dbg["dbg_kT"], kT_sb)
                    nc.sync.dma_start(dbg["dbg_v"], v_sb)
                    nc.sync.dma_start(dbg["dbg_ctxT"], ctxT_sb)

                # ---------------- Phase C: O-projection ----------------
                with tc.tile_pool(name="opsum", bufs=2, space="PSUM") as opsum:
                    for tt in range(T // P):
                        for nk in range(2):
                            ops = opsum.tile([P, 512], F32, tag="op", name=f"op{tt}_{nk}")
                            for cc in range(DT4):
                                nc.tensor.matmul(
                                    ops,
                                    _mm(ctxT_sb[:, cc, P * tt : P * (tt + 1)], OPROJ_DT),
                                    _mm(wo_sb[:, cc, 512 * nk : 512 * (nk + 1)], OPROJ_DT),
                                    start=(cc == 0),
                                    stop=(cc == DT4 - 1),
                                )
                            osb = tmppool.tile([P, 512], F32, tag="osb", name=f"osb{tt}_{nk}")
                            nc.scalar.activation(osb, ops, Act.Copy)
                            nc.sync.dma_start(
                                out[P * tt : P * (tt + 1), 512 * nk : 512 * (nk + 1)], osb
                            )

    nc.compile()
    return nc


def _masks():
    q = np.arange(P)[None, :] // BS
    k = np.arange(P)[:, None] // BS
    m = np.zeros((3, P, P), np.float32)
    m[0] = (q > k).astype(np.float32)    # strict (xt q vs x0 k, same tile)
    m[1] = (q >= k).astype(np.float32)   # incl (x0 q vs x0 k, same tile)
    m[2] = (q == k).astype(np.float32)   # diag (xt q vs xt k, same tile)
    return m


def kernel(x, Wq, bq, Wk, bk, Wv, bv, Wo, bo, block_size=4, **_):
    x = np.asarray(x, np.float32)
    Wq, bq = np.asarray(Wq, np.float32), np.asarray(bq, np.float32)
    Wk, bk = np.asarray(Wk, np.float32), np.asarray(bk, np.float32)
    Wv, bv = np.asarray(Wv, np.float32), np.asarray(bv, np.float32)
    Wo, bo = np.asarray(Wo, np.float32), np.asarray(bo, np.float32)

    if "nc" not in _CACHE:
        _CACHE["nc"] = _build()
    nc = _CACHE["nc"]

    masks = _masks()
    scale = HD ** -0.5
    in_maps = []
    for core in range(8):
        b, g = core // 2, core % 2
        cols = slice(DG * g, DG * (g + 1))
        in_maps.append(
            {
                "xT": np.ascontiguousarray(x[b].T),
                "wq": np.ascontiguousarray(Wq[:, cols]),
                "wk": np.ascontiguousarray(Wk[:, cols]),
                "wv": np.ascontiguousarray(Wv[:, cols]),
                "wo": np.ascontiguousarray(Wo[cols, :]),
                "bqs": np.ascontiguousarray(bq[cols]) * np.float32(scale),
                "bks": np.ascontiguousarray(bk[cols]),
                "msk": masks,
            }
        )

    _CACHE["last_in_maps"] = in_maps
    last_err = None
    for _attempt in range(4):
        try:
            res = run_bass_kernel_spmd(nc, in_maps, core_ids=list(range(8)), trace=False)
            break
        except Exception as e:  # transient NRT device flakes
            last_err = e
            if "UNRECOVERABLE" not in str(e) and "UNAVAILABLE" not in str(e):
                raise
            import time as _time

            import jax as _jax

            _time.sleep(5)
            try:
                _jax.clear_backends()
            except Exception:
                pass
    else:
        raise last_err

    corr = (bv @ Wo + bo).astype(np.float32)  # softmax rows sum to 1
    out = np.empty((B, T, D), np.float32)
    for b in range(B):
        out[b] = res.results[2 * b]["out"] + res.results[2 * b + 1]["out"] + corr
    return out


if __name__ == "__main__":
    rng = np.random.default_rng(0)
    inputs = {
        "x": rng.standard_normal((B, T, D), np.float32),
        "Wq": rng.standard_normal((D, D), np.float32) / 32,
        "bq": np.zeros(D, np.float32),
        "Wk": rng.standard_normal((D, D), np.float32) / 32,
        "bk": np.zeros(D, np.float32),
        "Wv": rng.standard_normal((D, D), np.float32) / 32,
        "bv": np.zeros(D, np.float32),
        "Wo": rng.standard_normal((D, D), np.float32) / 32,
        "bo": np.zeros(D, np.float32),
    }
    o = kernel(**inputs)
    print("ran", o.shape, o.dtype, float(np.abs(o).max()))


# revision 16
# speedup vs baseline: 1.0030x; 1.0030x over previous
"""BD3LM block-diffusion decoder layer on 8 trn2 NeuronCores.

Sharding: core = 2*b + g  (b = batch 0..3, g = head-group 0..1, 8 heads each).
Each core: QKV projections for its batch/head-group, sparse BD3LM attention
(only ~80 of 256 score tiles per head), O-projection against its Wo row-slice.
Host: sums the two group partials per batch and adds the (bv @ Wo + bo)
correction (softmax rows sum to 1, so the v-bias contributes exactly bv @ Wo).

Layouts on device (per core):
  qT/kT  [d_head_group=512, T=2048]  stored [128, 4, 2048]  (d on partitions)
  v      [T, 512] stored [128, 16, 8*65] with a per-head ones column -> the
         ctx matmul accumulates softmax denominators for free (row 64).
  scores computed transposed [k_tile=128, q_span] so softmax reduction is a
         PE matmul instead of a partition reduction; exp on ACT without
         max-subtraction (scores are ~N(0,1), bias-free overflow impossible);
         only 3 distinct 128x128 binary mask tiles (strict/incl/diag).
"""

import numpy as np

import concourse.bass as bass
import concourse.mybir as mybir
import concourse.tile as tile
from concourse import bacc
from concourse.bass_utils import run_bass_kernel_spmd

F32 = mybir.dt.float32
F32R = mybir.dt.float32r
Act = mybir.ActivationFunctionType

B, T, D = 4, 2048, 1024
H, HD = 16, 64
L = T // 2           # 1024, length of each of [xt | x0]
BS = 4               # block size
G = 2                # head groups (cores per batch)
DG = D // G          # 512 channels per group
HG = H // G          # 8 heads per core
P = 128
NT = L // P          # 8 key/query tiles per half
SLAB = 256           # projection t-slab width
KC = D // P          # 8 contraction chunks
DT4 = DG // P        # 4 output-partition tiles for qT/kT

# matmul dtype per family: float32 (exact, 4 cyc/row) or float32r (~1.5e-4, 1 cyc/row)
PROJ_DT = F32
ATTN_DT = F32
OPROJ_DT = F32
BCAST_DT = F32

REPEAT = 1  # loop whole computation inside the NEFF (timing experiments only)
DBG = False

_CACHE = {}


def _chunks512(a0, a1):
    """Split [a0, a1) at multiples of 512 (PSUM bank boundaries)."""
    out = []
    while a0 < a1:
        b1 = min(a1, (a0 // 512 + 1) * 512)
        out.append((a0, b1))
        a0 = b1
    return out


def _mm(ap, dt):
    return ap.bitcast(dt) if dt != F32 else ap


def _build():
    import concourse.tile_utils as tile_utils

    tile_utils.max_sbuf_usage = 204 * 1024  # trn2 has 208KB/partition usable

    nc = bacc.Bacc("TRN2", target_bir_lowering=False, debug=False, num_devices=8)
    dbg = {}
    if DBG:
        for nm, shp in (
            ("dbg_qT", [P, DT4, T]),
            ("dbg_kT", [P, DT4, T]),
            ("dbg_v", [P, T // P, HG * (HD + 1)]),
            ("dbg_ctxT", [P, DT4, T]),
            ("dbg_nd", [16, HD + 1, L]),
            ("dbg_at", [P, L]),
        ):
            dbg[nm] = nc.dram_tensor(nm, shp, F32, kind="ExternalOutput").ap()

    xT = nc.dram_tensor("xT", [D, T], F32, kind="ExternalInput").ap()
    wq = nc.dram_tensor("wq", [D, DG], F32, kind="ExternalInput").ap()
    wk = nc.dram_tensor("wk", [D, DG], F32, kind="ExternalInput").ap()
    wv = nc.dram_tensor("wv", [D, DG], F32, kind="ExternalInput").ap()
    wo = nc.dram_tensor("wo", [DG, D], F32, kind="ExternalInput").ap()
    bqs = nc.dram_tensor("bqs", [DG], F32, kind="ExternalInput").ap()
    bks = nc.dram_tensor("bks", [DG], F32, kind="ExternalInput").ap()
    msk = nc.dram_tensor("msk", [3, P, P], F32, kind="ExternalInput").ap()
    out = nc.dram_tensor("out", [T, D], F32, kind="ExternalOutput").ap()

    views = dict(
        xT_v=xT.rearrange("(kc p) t -> p kc t", p=P),    # [128, 8, 2048]
        wq_v=wq.rearrange("(kc p) m -> p kc m", p=P),    # [128, 8, 512]
        wk_v=wk.rearrange("(kc p) m -> p kc m", p=P),
        wv_v=wv.rearrange("(kc p) m -> p kc m", p=P),
        wo_v=wo.rearrange("(cc p) n -> p cc n", p=P),    # [128, 4, 1024]
        msk=msk,
        out=out,
    )

    with tile.TileContext(nc) as tc:
        with tc.tile_pool(name="persist", bufs=1) as pers:
            st = dict(
                qT_sb=pers.tile([P, DT4, T], F32, name="qT_sb"),
                kT_sb=pers.tile([P, DT4, T], F32, name="kT_sb"),
                v_sb=pers.tile([P, T // P, HG * (HD + 1)], F32, name="v_sb"),
                ctxT_sb=pers.tile([P, DT4, T], F32, name="ctxT_sb"),
                bq_sb=pers.tile([P, DT4], F32, name="bq_sb"),
                bk_sb=pers.tile([P, DT4], F32, name="bk_sb"),
            )
            nc.sync.dma_start(st["bq_sb"], bqs.rearrange("(c p) -> p c", p=P))
            nc.sync.dma_start(st["bk_sb"], bks.rearrange("(c p) -> p c", p=P))
            # ones columns for the softmax denominators
            ones_v = st["v_sb"].rearrange("p t (h c) -> p t h c", c=HD + 1)[
                :, :, :, HD : HD + 1
            ]
            nc.vector.memset(_mm(ones_v, ATTN_DT), 1.0)

            for _rep in range(REPEAT):
                _phases(nc, tc, dbg, st, views)

    nc.compile()
    return nc


def _phases(nc, tc, dbg, st, views):
    qT_sb, kT_sb, v_sb, ctxT_sb = (
        st["qT_sb"], st["kT_sb"], st["v_sb"], st["ctxT_sb"],
    )
    xT_v, wo_v, msk, out = views["xT_v"], views["wo_v"], views["msk"], views["out"]

    def _round(ap, dt):
        if dt == F32R:
            nc.vector.tensor_copy(ap.bitcast(F32R), ap)

    # ---------------- Phase A: QKV projections ----------------
    with (
        tc.tile_pool(name="wpool", bufs=1) as wpool,
        tc.tile_pool(name="xpool", bufs=2) as xpool,
        tc.tile_pool(name="ppsum", bufs=4, space="PSUM") as ppsum,
    ):
        wq_sb = wpool.tile([P, KC, DG], F32, name="wq_sb")
        wk_sb = wpool.tile([P, KC, DG], F32, name="wk_sb")
        wv_sb = wpool.tile([P, KC, DG], F32, name="wv_sb")
        nc.sync.dma_start(wq_sb, views["wq_v"])
        nc.sync.dma_start(wk_sb, views["wk_v"])
        nc.sync.dma_start(wv_sb, views["wv_v"])
        _round(wq_sb, PROJ_DT)
        _round(wk_sb, PROJ_DT)
        _round(wv_sb, PROJ_DT)

        for s in range(T // SLAB):
            x_sb = xpool.tile([P, KC, SLAB], F32, tag="x", name=f"x{s}")
            nc.sync.dma_start(x_sb, xT_v[:, :, SLAB * s : SLAB * (s + 1)])
            _round(x_sb, PROJ_DT)
            for w_sb, b_key, dst, scale in (
                (wq_sb, "bq_sb", qT_sb, HD ** -0.5),
                (wk_sb, "bk_sb", kT_sb, 1.0),
            ):
                for d4 in range(DT4):
                    ps = ppsum.tile([P, SLAB], F32, tag="pp", name=f"pp{s}_{d4}")
                    for kc in range(KC):
                        nc.tensor.matmul(
                            ps,
                            _mm(w_sb[:, kc, P * d4 : P * (d4 + 1)], PROJ_DT),
                            _mm(x_sb[:, kc, :], PROJ_DT),
                            start=(kc == 0),
                            stop=(kc == KC - 1),
                        )
                    nc.scalar.activation(
                        _mm(dst[:, d4, SLAB * s : SLAB * (s + 1)], ATTN_DT),
                        ps,
                        Act.Identity,
                        bias=st[b_key][:, d4 : d4 + 1],
                        scale=scale,
                    )
            for t2 in range(SLAB // P):
                tt = (SLAB // P) * s + t2
                ps = ppsum.tile([P, DG], F32, tag="ppv", name=f"ppv{tt}")
                for kc in range(KC):
                    nc.tensor.matmul(
                        ps,
                        _mm(x_sb[:, kc, P * t2 : P * (t2 + 1)], PROJ_DT),
                        _mm(wv_sb[:, kc, :], PROJ_DT),
                        start=(kc == 0),
                        stop=(kc == KC - 1),
                    )
                nc.vector.tensor_copy(
                    _mm(
                        v_sb[:, tt].rearrange("p (h c) -> p h c", c=HD + 1)[:, :, :HD],
                        ATTN_DT,
                    ),
                    ps.rearrange("p (h c) -> p h c", c=HD),
                )

    # ---------------- Phase B: sparse attention ----------------
    with (
        tc.tile_pool(name="apool", bufs=1) as apool,
        tc.tile_pool(name="atpool", bufs=6) as atpool,
        tc.tile_pool(name="tmppool", bufs=2) as tmppool,
        tc.tile_pool(name="spsum", bufs=2, space="PSUM") as spsum,
        tc.tile_pool(name="cpsum", bufs=2, space="PSUM") as cpsum,
    ):
        wo_sb = apool.tile([P, DT4, D], F32, name="wo_sb")
        nc.sync.dma_start(wo_sb, wo_v)
        _round(wo_sb, OPROJ_DT)
        m_strict = apool.tile([P, P], F32, name="m_strict")
        m_incl = apool.tile([P, P], F32, name="m_incl")
        m_diag = apool.tile([P, P], F32, name="m_diag")
        nc.sync.dma_start(m_strict, msk[0])
        nc.sync.dma_start(m_incl, msk[1])
        nc.sync.dma_start(m_diag, msk[2])
        ones_t = apool.tile([P, HD], F32, name="ones_t")  # row 64: K=1 bcast lhsT
        nc.vector.memset(_mm(ones_t, BCAST_DT), 1.0)

        for h in range(HG):
            c, p0 = h // 2, HD * (h % 2)
            qh = qT_sb[p0 : p0 + HD, c, :]   # [64, 2048]
            kh = kT_sb[p0 : p0 + HD, c, :]
            ctx_ps = [
                cpsum.tile([HD + 1, L], F32, tag="ctx", name=f"ctx{h}_{half}")
                for half in range(2)
            ]
            # last writer per (half, bank) sets stop=True:
            # xt half: diag matmuls finish each bank; x0 half: the j-loop does.
            for j in range(NT):
                kv = kh[:, L + P * j : L + P * (j + 1)]                  # [64, 128]
                vj = v_sb[:, NT + j, (HD + 1) * h : (HD + 1) * (h + 1)]  # [128, 65]
                for half in range(2):
                    mask = m_strict if half == 0 else m_incl
                    for a0, a1 in _chunks512(P * j, L):
                        n = a1 - a0
                        sc = spsum.tile(
                            [P, 512], F32, tag="sc", name=f"sc{h}_{j}_{half}_{a0}"
                        )[:, :n]
                        nc.tensor.matmul(
                            sc,
                            _mm(kv, ATTN_DT),
                            _mm(qh[:, L * half + a0 : L * half + a1], ATTN_DT),
                            start=True,
                            stop=True,
                        )
                        at = atpool.tile(
                            [P, 512], F32, tag="at", name=f"at{h}_{j}_{half}_{a0}"
                        )[:, :n]
                        nc.scalar.activation(_mm(at, ATTN_DT), sc, Act.Exp)
                        if a0 == P * j:
                            nc.vector.tensor_mul(
                                _mm(at[:, :P], ATTN_DT), at[:, :P], mask
                            )
                        if DBG and h == 0 and j == 0 and half == 1:
                            nc.sync.dma_start(dbg["dbg_at"][:, a0:a1], at)
                        # x0 half: stop on the last j touching this bank
                        last = half == 1 and (
                            (a1 <= 512 and j == 3) or (a0 >= 512 and j == NT - 1)
                        )
                        nc.tensor.matmul(
                            ctx_ps[half][:, a0:a1],
                            _mm(vj, ATTN_DT),
                            _mm(at, ATTN_DT),
                            start=(j == 0),
                            stop=last,
                        )
            # xt-xt block-diagonal tiles
            for i in range(NT):
                scd = spsum.tile([P, 512], F32, tag="sc", name=f"scd{h}_{i}")[:, :P]
                nc.tensor.matmul(
                    scd,
                    _mm(kh[:, P * i : P * (i + 1)], ATTN_DT),
                    _mm(qh[:, P * i : P * (i + 1)], ATTN_DT),
                    start=True,
                    stop=True,
                )
                atd = atpool.tile([P, 512], F32, tag="at", name=f"atd{h}_{i}")[:, :P]
                nc.scalar.activation(_mm(atd, ATTN_DT), scd, Act.Exp)
                nc.vector.tensor_mul(_mm(atd, ATTN_DT), atd, m_diag)
                nc.tensor.matmul(
                    ctx_ps[0][:, P * i : P * (i + 1)],
                    _mm(v_sb[:, i, (HD + 1) * h : (HD + 1) * (h + 1)], ATTN_DT),
                    _mm(atd, ATTN_DT),
                    start=False,
                    stop=(i == 3 or i == NT - 1),
                )
            if DBG:
                for half in range(2):
                    ndc = tmppool.tile(
                        [HD + 1, L], F32, tag="ndc", name=f"ndc{h}_{half}"
                    )
                    nc.scalar.activation(ndc, ctx_ps[half], Act.Copy)
                    nc.sync.dma_start(dbg["dbg_nd"][2 * h + half], ndc)
            # normalize: ctxT = ctx[:64] * (1 / denom), denom = row 64
            for half in range(2):
                recip = tmppool.tile([P, L], F32, tag="recip", name=f"rc{h}_{half}")
                nc.vector.reciprocal(
                    recip[HD : HD + 1, :], ctx_ps[half][HD : HD + 1, :]
                )
                _round(recip[HD : HD + 1, :], BCAST_DT)
                rb = tmppool.tile([HD, L], F32, tag="rb", name=f"rb{h}_{half}")
                # PE broadcast: ones[1,64].T @ recip[1,n] -> [64, n]
                for c0 in range(0, L, 512):
                    bc = spsum.tile(
                        [P, 512], F32, tag="sc", name=f"bc{h}_{half}_{c0}"
                    )[:HD, :]
                    nc.tensor.matmul(
                        bc,
                        _mm(ones_t[HD : HD + 1, :], BCAST_DT),
                        _mm(recip[HD : HD + 1, c0 : c0 + 512], BCAST_DT),
                        start=True,
                        stop=True,
                    )
                    nc.scalar.activation(rb[:, c0 : c0 + 512], bc, Act.Copy)
                if h % 2 == 0:
                    nc.vector.tensor_mul(
                        ctxT_sb[:HD, c, L * half : L * (half + 1)],
                        ctx_ps[half][:HD, :],
                        rb,
                    )
                else:
                    cs = tmppool.tile([HD, L], F32, tag="cs", name=f"cs{h}_{half}")
                    nc.vector.tensor_mul(cs, ctx_ps[half][:HD, :], rb)
                    nc.sync.dma_start(
                        ctxT_sb[HD : 2 * HD, c, L * half : L * (half + 1)], cs
                    )
            if h % 2 == 1:
                _round(ctxT_sb[:, c, :], OPROJ_DT)

        if DBG:
            nc.sync.dma_start(dbg["dbg_qT"], qT_sb)
            nc.sync.dma_start(dbg["dbg_kT"], kT_sb)
            nc.sync.dma_start(dbg["dbg_v"], v_sb)
            nc.sync.dma_start(dbg["dbg_ctxT"], ctxT_sb)

        # ---------------- Phase C: O-projection ----------------
        with tc.tile_pool(name="opsum", bufs=2, space="PSUM") as opsum:
            for tt in range(T // P):
                for nk in range(2):
                    ops = opsum.tile([P, 512], F32, tag="op", name=f"op{tt}_{nk}")
                    for cc in range(DT4):
                        nc.tensor.matmul(
                            ops,
                            _mm(ctxT_sb[:, cc, P * tt : P * (tt + 1)], OPROJ_DT),
                            _mm(wo_sb[:, cc, 512 * nk : 512 * (nk + 1)], OPROJ_DT),
                            start=(cc == 0),
                            stop=(cc == DT4 - 1),
                        )
                    osb = tmppool.tile([P, 512], F32, tag="osb", name=f"osb{tt}_{nk}")
                    nc.scalar.activation(osb, ops, Act.Copy)
                    nc.sync.dma_start(
                        out[P * tt : P * (tt + 1), 512 * nk : 512 * (nk + 1)], osb
                    )


def _masks():
    q = np.arange(P)[None, :] // BS
    k = np.arange(P)[:, None] // BS
    m = np.zeros((3, P, P), np.float32)
    m[0] = (q > k).astype(np.float32)    # strict (xt q vs x0 k, same tile)
    m[1] = (q >= k).astype(np.float32)   # incl (x0 q vs x0 k, same tile)
    m[2] = (q == k).astype(np.float32)   # diag (xt q vs xt k, same tile)
    return m


def kernel(x, Wq, bq, Wk, bk, Wv, bv, Wo, bo, block_size=4, **_):
    x = np.asarray(x, np.float32)
    Wq, bq = np.asarray(Wq, np.float32), np.asarray(bq, np.float32)
    Wk, bk = np.asarray(Wk, np.float32), np.asarray(bk, np.float32)
    Wv, bv = np.asarray(Wv, np.float32), np.asarray(bv, np.float32)
    Wo, bo = np.asarray(Wo, np.float32), np.asarray(bo, np.float32)

    if "nc" not in _CACHE:
        _CACHE["nc"] = _build()
    nc = _CACHE["nc"]

    masks = _masks()
    scale = HD ** -0.5
    in_maps = []
    for core in range(8):
        b, g = core // 2, core % 2
        cols = slice(DG * g, DG * (g + 1))
        in_maps.append(
            {
                "xT": np.ascontiguousarray(x[b].T),
                "wq": np.ascontiguousarray(Wq[:, cols]),
                "wk": np.ascontiguousarray(Wk[:, cols]),
                "wv": np.ascontiguousarray(Wv[:, cols]),
                "wo": np.ascontiguousarray(Wo[cols, :]),
                "bqs": np.ascontiguousarray(bq[cols]) * np.float32(scale),
                "bks": np.ascontiguousarray(bk[cols]),
                "msk": masks,
            }
        )

    _CACHE["last_in_maps"] = in_maps
    last_err = None
    for _attempt in range(4):
        try:
            res = run_bass_kernel_spmd(nc, in_maps, core_ids=list(range(8)), trace=False)
            break
        except Exception as e:  # transient NRT device flakes
            last_err = e
            if "UNRECOVERABLE" not in str(e) and "UNAVAILABLE" not in str(e):
                raise
            import time as _time

            import jax as _jax

            _time.sleep(5)
            try:
                _jax.clear_backends()
            except Exception:
                pass
    else:
        raise last_err

    corr = (bv @ Wo + bo).astype(np.float32)  # softmax rows sum to 1
    out = np.empty((B, T, D), np.float32)
    for b in range(B):
        out[b] = res.results[2 * b]["out"] + res.results[2 * b + 1]["out"] + corr
    return out


if __name__ == "__main__":
    rng = np.random.default_rng(0)
    inputs = {
        "x": rng.standard_normal((B, T, D)).astype(np.float32),
        "Wq": (rng.standard_normal((D, D)) / 32).astype(np.float32),
        "bq": np.zeros(D, np.float32),
        "Wk": (rng.standard_normal((D, D)) / 32).astype(np.float32),
        "bk": np.zeros(D, np.float32),
        "Wv": (rng.standard_normal((D, D)) / 32).astype(np.float32),
        "bv": np.zeros(D, np.float32),
        "Wo": (rng.standard_normal((D, D)) / 32).astype(np.float32),
        "bo": np.zeros(D, np.float32),
    }
    o = kernel(**inputs)
    print("ran", o.shape, o.dtype, float(np.abs(o).max()))


# revision 23
# speedup vs baseline: 1.0590x; 1.0558x over previous
"""BD3LM block-diffusion decoder layer on 8 trn2 NeuronCores.

Sharding: core = 2*b + g  (b = batch 0..3, g = head-group 0..1, 8 heads each).
Each core: QKV projections for its batch/head-group, sparse BD3LM attention
(only ~80 of 256 score tiles per head), O-projection against its Wo row-slice.
Host: sums the two group partials per batch and adds the (bv @ Wo + bo)
correction (softmax rows sum to 1, so the v-bias contributes exactly bv @ Wo).

Layouts on device (per core):
  qT/kT  [d_head_group=512, T=2048]  stored [128, 4, 2048]  (d on partitions)
  v      [T, 512] stored [128, 16, 8*65] with a per-head ones column -> the
         ctx matmul accumulates softmax denominators for free (row 64).
  scores computed transposed [k_tile=128, q_span] so softmax reduction is a
         PE matmul instead of a partition reduction; exp on ACT without
         max-subtraction (scores are ~N(0,1), bias-free overflow impossible);
         only 3 distinct 128x128 binary mask tiles (strict/incl/diag).
"""

import numpy as np

import concourse.bass as bass
import concourse.mybir as mybir
import concourse.tile as tile
from concourse import bacc
from concourse.bass_utils import run_bass_kernel_spmd

F32 = mybir.dt.float32
F32R = mybir.dt.float32r
Act = mybir.ActivationFunctionType

B, T, D = 4, 2048, 1024
H, HD = 16, 64
L = T // 2           # 1024, length of each of [xt | x0]
BS = 4               # block size
G = 2                # head groups (cores per batch)
DG = D // G          # 512 channels per group
HG = H // G          # 8 heads per core
P = 128
NT = L // P          # 8 key/query tiles per half
SLAB = 256           # projection t-slab width
KC = D // P          # 8 contraction chunks
DT4 = DG // P        # 4 output-partition tiles for qT/kT

# matmul dtype per family: float32 (exact, 4 cyc/row) or float32r (~2.7e-4
# end-to-end, 1 cyc/row at N>=256). f32r measured ~2.2x faster end-to-end.
PROJ_DT = F32R
ATTN_DT = F32R
OPROJ_DT = F32R
BCAST_DT = F32R

REPEAT = 1  # loop whole computation inside the NEFF (timing experiments only)
DBG = False

_CACHE = {}


def _chunks512(a0, a1):
    """Split [a0, a1) at multiples of 512 (PSUM bank boundaries)."""
    out = []
    while a0 < a1:
        b1 = min(a1, (a0 // 512 + 1) * 512)
        out.append((a0, b1))
        a0 = b1
    return out


def _mm(ap, dt):
    return ap.bitcast(dt) if dt != F32 else ap


def _build():
    import concourse.tile_utils as tile_utils

    tile_utils.max_sbuf_usage = 204 * 1024  # trn2 has 208KB/partition usable

    nc = bacc.Bacc("TRN2", target_bir_lowering=False, debug=False, num_devices=8)
    dbg = {}
    if DBG:
        for nm, shp in (
            ("dbg_qT", [P, DT4, T]),
            ("dbg_kT", [P, DT4, T]),
            ("dbg_v", [P, T // P, HG * (HD + 1)]),
            ("dbg_ctxT", [P, DT4, T]),
            ("dbg_nd", [16, HD + 1, L]),
            ("dbg_at", [P, L]),
        ):
            dbg[nm] = nc.dram_tensor(nm, shp, F32, kind="ExternalOutput").ap()

    xT = nc.dram_tensor("xT", [D, T], F32, kind="ExternalInput").ap()
    wq = nc.dram_tensor("wq", [D, DG], F32, kind="ExternalInput").ap()
    wk = nc.dram_tensor("wk", [D, DG], F32, kind="ExternalInput").ap()
    wv = nc.dram_tensor("wv", [D, DG], F32, kind="ExternalInput").ap()
    wo = nc.dram_tensor("wo", [DG, D], F32, kind="ExternalInput").ap()
    bqs = nc.dram_tensor("bqs", [DG], F32, kind="ExternalInput").ap()
    bks = nc.dram_tensor("bks", [DG], F32, kind="ExternalInput").ap()
    msk = nc.dram_tensor("msk", [3, P, P], F32, kind="ExternalInput").ap()
    out = nc.dram_tensor("out", [T, D], F32, kind="ExternalOutput").ap()

    views = dict(
        xT_v=xT.rearrange("(kc p) t -> p kc t", p=P),    # [128, 8, 2048]
        wq_v=wq.rearrange("(kc p) m -> p kc m", p=P),    # [128, 8, 512]
        wk_v=wk.rearrange("(kc p) m -> p kc m", p=P),
        wv_v=wv.rearrange("(kc p) m -> p kc m", p=P),
        wo_v=wo.rearrange("(cc p) n -> p cc n", p=P),    # [128, 4, 1024]
        msk=msk,
        out=out,
    )

    with tile.TileContext(nc) as tc:
        with tc.tile_pool(name="persist", bufs=1) as pers:
            st = dict(
                qT_sb=pers.tile([P, DT4, T], F32, name="qT_sb"),
                kT_sb=pers.tile([P, DT4, T], F32, name="kT_sb"),
                v_sb=pers.tile([P, T // P, HG * (HD + 1)], F32, name="v_sb"),
                bq_sb=pers.tile([P, DT4], F32, name="bq_sb"),
                bk_sb=pers.tile([P, DT4], F32, name="bk_sb"),
            )
            nc.sync.dma_start(st["bq_sb"], bqs.rearrange("(c p) -> p c", p=P))
            nc.sync.dma_start(st["bk_sb"], bks.rearrange("(c p) -> p c", p=P))
            # ones columns for the softmax denominators
            ones_c = pers.tile([P, 1], F32, name="ones_c")
            nc.vector.memset(ones_c, 1.0)
            ones_v = st["v_sb"].rearrange("p t (h c) -> p (t h) c", c=HD + 1)[
                :, :, HD : HD + 1
            ]
            if ATTN_DT == F32:
                nc.vector.memset(ones_v, 1.0)
            else:
                nc.vector.tensor_copy(
                    _mm(ones_v, ATTN_DT),
                    ones_c[:, 0:1, None].to_broadcast(tuple(ones_v.shape)),
                )
            st["ones_c"] = ones_c

            for _rep in range(REPEAT):
                _phases(nc, tc, dbg, st, views)

    nc.compile()
    return nc


def _phases(nc, tc, dbg, st, views):
    qT_sb, kT_sb, v_sb = st["qT_sb"], st["kT_sb"], st["v_sb"]
    xT_v, wo_v, msk, out = views["xT_v"], views["wo_v"], views["msk"], views["out"]

    # ---------------- Phase A: QKV projections (one x stream) ----------------
    with (
        tc.tile_pool(name="wpool", bufs=1) as wpool,
        tc.tile_pool(name="xpool", bufs=2) as xpool,
        tc.tile_pool(name="ppsum", bufs=4, space="PSUM") as ppsum,
        tc.tile_pool(name="vpsum", bufs=4, space="PSUM") as vpsum,
    ):
        wq_sb = wpool.tile([P, KC, DG], F32, name="wq_sb")
        wk_sb = wpool.tile([P, KC, DG], F32, name="wk_sb")
        wv_sb = wpool.tile([P, KC, DG], F32, name="wv_sb")
        nc.sync.dma_start(_mm(wq_sb, PROJ_DT), _mm(views["wq_v"], PROJ_DT))
        nc.sync.dma_start(_mm(wk_sb, PROJ_DT), _mm(views["wk_v"], PROJ_DT))
        nc.sync.dma_start(_mm(wv_sb, PROJ_DT), _mm(views["wv_v"], PROJ_DT))
        for s in range(T // 512):
            x_sb = xpool.tile([P, KC, 512], F32, tag="x", name=f"x{s}")
            nc.sync.dma_start(
                _mm(x_sb, PROJ_DT), _mm(xT_v[:, :, 512 * s : 512 * (s + 1)], PROJ_DT)
            )
            for w_sb, b_key, dst, scale in (
                (wq_sb, "bq_sb", qT_sb, HD ** -0.5),
                (wk_sb, "bk_sb", kT_sb, 1.0),
            ):
                for d4 in range(DT4):
                    ps = ppsum.tile([P, 512], F32, tag="pp", name=f"pp{s}_{d4}")
                    for kc in range(KC):
                        nc.tensor.matmul(
                            ps,
                            _mm(w_sb[:, kc, P * d4 : P * (d4 + 1)], PROJ_DT),
                            _mm(x_sb[:, kc, :], PROJ_DT),
                            start=(kc == 0),
                            stop=(kc == KC - 1),
                        )
                    nc.scalar.activation(
                        _mm(dst[:, d4, 512 * s : 512 * (s + 1)], ATTN_DT),
                        ps,
                        Act.Identity,
                        bias=st[b_key][:, d4 : d4 + 1],
                        scale=scale,
                    )
            for t2 in range(4):
                tt = 4 * s + t2
                ps = vpsum.tile([P, DG], F32, tag="ppv", name=f"ppv{tt}")
                for kc in range(KC):
                    nc.tensor.matmul(
                        ps,
                        _mm(x_sb[:, kc, P * t2 : P * (t2 + 1)], PROJ_DT),
                        _mm(wv_sb[:, kc, :], PROJ_DT),
                        start=(kc == 0),
                        stop=(kc == KC - 1),
                    )
                nc.vector.tensor_copy(
                    _mm(
                        v_sb[:, tt].rearrange("p (h c) -> p h c", c=HD + 1)[:, :, :HD],
                        ATTN_DT,
                    ),
                    ps.rearrange("p (h c) -> p h c", c=HD),
                )

    # ---------------- Phase B: sparse attention ----------------
    with (
        tc.tile_pool(name="apool", bufs=1) as apool,
        tc.tile_pool(name="atpool", bufs=6) as atpool,
        tc.tile_pool(name="tmppool", bufs=2) as tmppool,
    ):
        ctxT_sb = apool.tile([P, DT4, T], F32, name="ctxT_sb")
        wo_sb = apool.tile([P, DT4, D], F32, name="wo_sb")
        nc.sync.dma_start(_mm(wo_sb, OPROJ_DT), _mm(wo_v, OPROJ_DT))
        m_strict = apool.tile([P, P], F32, name="m_strict")
        m_incl = apool.tile([P, P], F32, name="m_incl")
        m_diag = apool.tile([P, P], F32, name="m_diag")
        nc.sync.dma_start(m_strict, msk[0])
        nc.sync.dma_start(m_incl, msk[1])
        nc.sync.dma_start(m_diag, msk[2])
        ones_t = apool.tile([P, HD], F32, name="ones_t")  # row 64: K=1 bcast lhsT
        if BCAST_DT == F32:
            nc.vector.memset(ones_t, 1.0)
        else:
            nc.vector.tensor_copy(
                _mm(ones_t, BCAST_DT),
                st["ones_c"][:, 0:1].to_broadcast((P, HD)),
            )

        from contextlib import ExitStack as _ES

        _es = _ES()
        spsum = _es.enter_context(tc.tile_pool(name="spsum", bufs=4, space="PSUM"))
        cpsum = _es.enter_context(tc.tile_pool(name="cpsum", bufs=2, space="PSUM"))
        for h in range(HG):
            c, p0 = h // 2, HD * (h % 2)
            qh = qT_sb[p0 : p0 + HD, c, :]   # [64, 2048]
            kh = kT_sb[p0 : p0 + HD, c, :]
            for half in range(2):
                ctx = cpsum.tile([HD + 1, L], F32, tag="ctx", name=f"ctx{h}_{half}")
                mask = m_strict if half == 0 else m_incl
                for j in range(NT):
                    kv = kh[:, L + P * j : L + P * (j + 1)]                  # [64, 128]
                    vj = v_sb[:, NT + j, (HD + 1) * h : (HD + 1) * (h + 1)]  # [128, 65]
                    for a0, a1 in _chunks512(P * j, L):
                        n = a1 - a0
                        sc = spsum.tile(
                            [P, 512], F32, tag="sc", name=f"sc{h}_{j}_{half}_{a0}"
                        )[:, :n]
                        nc.tensor.matmul(
                            sc,
                            _mm(kv, ATTN_DT),
                            _mm(qh[:, L * half + a0 : L * half + a1], ATTN_DT),
                            start=True,
                            stop=True,
                        )
                        at = atpool.tile(
                            [P, 512], F32, tag="at", name=f"at{h}_{j}_{half}_{a0}"
                        )[:, :n]
                        nc.scalar.activation(_mm(at, ATTN_DT), sc, Act.Exp)
                        if a0 == P * j:
                            nc.vector.tensor_mul(
                                _mm(at[:, :P], ATTN_DT), at[:, :P], mask
                            )
                        if DBG and h == 0 and j == 0 and half == 1:
                            nc.sync.dma_start(dbg["dbg_at"][:, a0:a1], at)
                        # x0 half: stop on the last j touching this bank
                        last = half == 1 and (
                            (a1 <= 512 and j == 3) or (a0 >= 512 and j == NT - 1)
                        )
                        nc.tensor.matmul(
                            ctx[:, a0:a1],
                            _mm(vj, ATTN_DT),
                            _mm(at, ATTN_DT),
                            start=(j == 0),
                            stop=last,
                        )
                if half == 0:
                    # xt-xt block-diagonal tiles
                    for i in range(NT):
                        scd = spsum.tile(
                            [P, 512], F32, tag="sc", name=f"scd{h}_{i}"
                        )[:, :P]
                        nc.tensor.matmul(
                            scd,
                            _mm(kh[:, P * i : P * (i + 1)], ATTN_DT),
                            _mm(qh[:, P * i : P * (i + 1)], ATTN_DT),
                            start=True,
                            stop=True,
                        )
                        atd = atpool.tile(
                            [P, 512], F32, tag="at", name=f"atd{h}_{i}"
                        )[:, :P]
                        nc.scalar.activation(_mm(atd, ATTN_DT), scd, Act.Exp)
                        nc.vector.tensor_mul(_mm(atd, ATTN_DT), atd, m_diag)
                        nc.tensor.matmul(
                            ctx[:, P * i : P * (i + 1)],
                            _mm(v_sb[:, i, (HD + 1) * h : (HD + 1) * (h + 1)], ATTN_DT),
                            _mm(atd, ATTN_DT),
                            start=False,
                            stop=(i == 3 or i == NT - 1),
                        )
                if DBG:
                    ndc = tmppool.tile(
                        [HD + 1, L], F32, tag="ndc", name=f"ndc{h}_{half}"
                    )
                    nc.scalar.activation(ndc, ctx, Act.Copy)
                    nc.sync.dma_start(dbg["dbg_nd"][2 * h + half], ndc)
                # normalize: ctxT = ctx[:64] * (1 / denom), denom = row 64
                recip = tmppool.tile([P, L], F32, tag="recip", name=f"rc{h}_{half}")
                with nc.allow_low_precision(reason="deliberate f32r rounding"):
                    nc.vector.reciprocal(
                        _mm(recip[HD : HD + 1, :], BCAST_DT),
                        ctx[HD : HD + 1, :],
                    )
                rb = tmppool.tile([HD, L], F32, tag="rb", name=f"rb{h}_{half}")
                # PE broadcast: ones[1,64].T @ recip[1,n] -> [64, n]
                for c0 in range(0, L, 512):
                    bc = spsum.tile(
                        [P, 512], F32, tag="sc", name=f"bc{h}_{half}_{c0}"
                    )[:HD, :]
                    nc.tensor.matmul(
                        bc,
                        _mm(ones_t[HD : HD + 1, :], BCAST_DT),
                        _mm(recip[HD : HD + 1, c0 : c0 + 512], BCAST_DT),
                        start=True,
                        stop=True,
                    )
                    nc.vector.tensor_copy(rb[:, c0 : c0 + 512], bc)
                if h % 2 == 0:
                    nc.vector.tensor_mul(
                        _mm(ctxT_sb[:HD, c, L * half : L * (half + 1)], OPROJ_DT),
                        ctx[:HD, :],
                        rb,
                    )
                else:
                    cs = tmppool.tile([HD, L], F32, tag="cs", name=f"cs{h}_{half}")
                    nc.vector.tensor_mul(_mm(cs, OPROJ_DT), ctx[:HD, :], rb)
                    nc.sync.dma_start(
                        _mm(ctxT_sb[HD : 2 * HD, c, L * half : L * (half + 1)], OPROJ_DT),
                        _mm(cs, OPROJ_DT),
                    )

        if DBG:
            nc.sync.dma_start(dbg["dbg_qT"], qT_sb)
            nc.sync.dma_start(dbg["dbg_kT"], kT_sb)
            nc.sync.dma_start(dbg["dbg_v"], v_sb)
            nc.sync.dma_start(dbg["dbg_ctxT"], ctxT_sb)

        _es.close()

        # ---------------- Phase C: O-projection ----------------
        with tc.tile_pool(name="opsum", bufs=6, space="PSUM") as opsum:
            for tt in range(T // P):
                for nk in range(2):
                    ops = opsum.tile([P, 512], F32, tag="op", name=f"op{tt}_{nk}")
                    for cc in range(DT4):
                        nc.tensor.matmul(
                            ops,
                            _mm(ctxT_sb[:, cc, P * tt : P * (tt + 1)], OPROJ_DT),
                            _mm(wo_sb[:, cc, 512 * nk : 512 * (nk + 1)], OPROJ_DT),
                            start=(cc == 0),
                            stop=(cc == DT4 - 1),
                        )
                    osb = tmppool.tile([P, 512], F32, tag="osb", name=f"osb{tt}_{nk}")
                    nc.vector.tensor_copy(osb, ops)
                    nc.sync.dma_start(
                        out[P * tt : P * (tt + 1), 512 * nk : 512 * (nk + 1)], osb
                    )


def _masks():
    q = np.arange(P)[None, :] // BS
    k = np.arange(P)[:, None] // BS
    m = np.zeros((3, P, P), np.float32)
    m[0] = (q > k).astype(np.float32)    # strict (xt q vs x0 k, same tile)
    m[1] = (q >= k).astype(np.float32)   # incl (x0 q vs x0 k, same tile)
    m[2] = (q == k).astype(np.float32)   # diag (xt q vs xt k, same tile)
    return m


def kernel(x, Wq, bq, Wk, bk, Wv, bv, Wo, bo, block_size=4, **_):
    x = np.asarray(x, np.float32)
    Wq, bq = np.asarray(Wq, np.float32), np.asarray(bq, np.float32)
    Wk, bk = np.asarray(Wk, np.float32), np.asarray(bk, np.float32)
    Wv, bv = np.asarray(Wv, np.float32), np.asarray(bv, np.float32)
    Wo, bo = np.asarray(Wo, np.float32), np.asarray(bo, np.float32)

    if "nc" not in _CACHE:
        _CACHE["nc"] = _build()
    nc = _CACHE["nc"]

    masks = _masks()
    scale = HD ** -0.5
    in_maps = []
    for core in range(8):
        b, g = core // 2, core % 2
        cols = slice(DG * g, DG * (g + 1))
        in_maps.append(
            {
                "xT": np.ascontiguousarray(x[b].T),
                "wq": np.ascontiguousarray(Wq[:, cols]),
                "wk": np.ascontiguousarray(Wk[:, cols]),
                "wv": np.ascontiguousarray(Wv[:, cols]),
                "wo": np.ascontiguousarray(Wo[cols, :]),
                "bqs": np.ascontiguousarray(bq[cols]) * np.float32(scale),
                "bks": np.ascontiguousarray(bk[cols]),
                "msk": masks,
            }
        )

    _CACHE["last_in_maps"] = in_maps
    last_err = None
    for _attempt in range(4):
        try:
            res = run_bass_kernel_spmd(nc, in_maps, core_ids=list(range(8)), trace=False)
            break
        except Exception as e:  # transient NRT device flakes
            last_err = e
            if "UNRECOVERABLE" not in str(e) and "UNAVAILABLE" not in str(e):
                raise
            import time as _time

            import jax as _jax

            _time.sleep(5)
            try:
                _jax.clear_backends()
            except Exception:
                pass
    else:
        raise last_err

    corr = (bv @ Wo + bo).astype(np.float32)  # softmax rows sum to 1
    out = np.empty((B, T, D), np.float32)
    for b in range(B):
        out[b] = res.results[2 * b]["out"] + res.results[2 * b + 1]["out"] + corr
    return out


if __name__ == "__main__":
    rng = np.random.default_rng(0)
    inputs = {
        "x": rng.standard_normal((B, T, D)).astype(np.float32),
        "Wq": (rng.standard_normal((D, D)) / 32).astype(np.float32),
        "bq": np.zeros(D, np.float32),
        "Wk": (rng.standard_normal((D, D)) / 32).astype(np.float32),
        "bk": np.zeros(D, np.float32),
        "Wv": (rng.standard_normal((D, D)) / 32).astype(np.float32),
        "bv": np.zeros(D, np.float32),
        "Wo": (rng.standard_normal((D, D)) / 32).astype(np.float32),
        "bo": np.zeros(D, np.float32),
    }
    o = kernel(**inputs)
    print("ran", o.shape, o.dtype, float(np.abs(o).max()))


# revision 24
# speedup vs baseline: 48.6829x; 45.9694x over previous
"""BD3LM block-diffusion decoder layer on 8 trn2 NeuronCores.

Sharding: core = 2*b + g  (b = batch 0..3, g = head-group 0..1, 8 heads each).
Each core: QKV projections for its batch/head-group, sparse BD3LM attention
(only ~80 of 256 score tiles per head), O-projection against its Wo row-slice.
Host: sums the two group partials per batch and adds the (bv @ Wo + bo)
correction (softmax rows sum to 1, so the v-bias contributes exactly bv @ Wo).

Layouts on device (per core):
  qT/kT  [d_head_group=512, T=2048]  stored [128, 4, 2048]  (d on partitions)
  v      [T, 512] stored [128, 16, 8*65] with a per-head ones column -> the
         ctx matmul accumulates softmax denominators for free (row 64).
  scores computed transposed [k_tile=128, q_span] so softmax reduction is a
         PE matmul instead of a partition reduction; exp on ACT without
         max-subtraction (scores are ~N(0,1), bias-free overflow impossible);
         only 3 distinct 128x128 binary mask tiles (strict/incl/diag).
"""

import numpy as np

import concourse.bass as bass
import concourse.mybir as mybir
import concourse.tile as tile
from concourse import bacc
from concourse.bass_utils import run_bass_kernel_spmd

F32 = mybir.dt.float32
F32R = mybir.dt.float32r
Act = mybir.ActivationFunctionType

B, T, D = 4, 2048, 1024
H, HD = 16, 64
L = T // 2           # 1024, length of each of [xt | x0]
BS = 4               # block size
G = 2                # head groups (cores per batch)
DG = D // G          # 512 channels per group
HG = H // G          # 8 heads per core
P = 128
NT = L // P          # 8 key/query tiles per half
SLAB = 256           # projection t-slab width
KC = D // P          # 8 contraction chunks
DT4 = DG // P        # 4 output-partition tiles for qT/kT

# matmul dtype per family: float32 (exact, 4 cyc/row) or float32r (~2.7e-4
# end-to-end, 1 cyc/row at N>=256). f32r measured ~2.2x faster end-to-end.
PROJ_DT = F32R
ATTN_DT = F32R
OPROJ_DT = F32R
BCAST_DT = F32R

REPEAT = 1  # loop whole computation inside the NEFF (timing experiments only)
DBG = False

_CACHE = {}


def _chunks512(a0, a1):
    """Split [a0, a1) at multiples of 512 (PSUM bank boundaries)."""
    out = []
    while a0 < a1:
        b1 = min(a1, (a0 // 512 + 1) * 512)
        out.append((a0, b1))
        a0 = b1
    return out


def _mm(ap, dt):
    return ap.bitcast(dt) if dt != F32 else ap


def _build():
    import concourse.tile_utils as tile_utils

    tile_utils.max_sbuf_usage = 204 * 1024  # trn2 has 208KB/partition usable

    nc = bacc.Bacc("TRN2", target_bir_lowering=False, debug=False, num_devices=8)
    dbg = {}
    if DBG:
        for nm, shp in (
            ("dbg_qT", [P, DT4, T]),
            ("dbg_kT", [P, DT4, T]),
            ("dbg_v", [P, T // P, HG * (HD + 1)]),
            ("dbg_ctxT", [P, DT4, T]),
            ("dbg_nd", [16, HD + 1, L]),
            ("dbg_at", [P, L]),
        ):
            dbg[nm] = nc.dram_tensor(nm, shp, F32, kind="ExternalOutput").ap()

    xT = nc.dram_tensor("xT", [D, T], F32, kind="ExternalInput").ap()
    wq = nc.dram_tensor("wq", [D, DG], F32, kind="ExternalInput").ap()
    wk = nc.dram_tensor("wk", [D, DG], F32, kind="ExternalInput").ap()
    wv = nc.dram_tensor("wv", [D, DG], F32, kind="ExternalInput").ap()
    wo = nc.dram_tensor("wo", [DG, D], F32, kind="ExternalInput").ap()
    bqs = nc.dram_tensor("bqs", [DG], F32, kind="ExternalInput").ap()
    bks = nc.dram_tensor("bks", [DG], F32, kind="ExternalInput").ap()
    msk = nc.dram_tensor("msk", [3, P, P], F32, kind="ExternalInput").ap()
    out = nc.dram_tensor("out", [T, D], F32, kind="ExternalOutput").ap()

    views = dict(
        xT_v=xT.rearrange("(kc p) t -> p kc t", p=P),    # [128, 8, 2048]
        wq_v=wq.rearrange("(kc p) m -> p kc m", p=P),    # [128, 8, 512]
        wk_v=wk.rearrange("(kc p) m -> p kc m", p=P),
        wv_v=wv.rearrange("(kc p) m -> p kc m", p=P),
        wo_v=wo.rearrange("(cc p) n -> p cc n", p=P),    # [128, 4, 1024]
        msk=msk,
        out=out,
    )

    with tile.TileContext(nc) as tc:
        with tc.tile_pool(name="persist", bufs=1) as pers:
            st = dict(
                qT_sb=pers.tile([P, DT4, T], F32, name="qT_sb"),
                kT_sb=pers.tile([P, DT4, T], F32, name="kT_sb"),
                v_sb=pers.tile([P, T // P, HG * (HD + 1)], F32, name="v_sb"),
                bq_sb=pers.tile([P, DT4], F32, name="bq_sb"),
                bk_sb=pers.tile([P, DT4], F32, name="bk_sb"),
            )
            nc.sync.dma_start(st["bq_sb"], bqs.rearrange("(c p) -> p c", p=P))
            nc.sync.dma_start(st["bk_sb"], bks.rearrange("(c p) -> p c", p=P))
            # ones columns for the softmax denominators
            ones_c = pers.tile([P, 1], F32, name="ones_c")
            nc.vector.memset(ones_c, 1.0)
            ones_v = st["v_sb"].rearrange("p t (h c) -> p (t h) c", c=HD + 1)[
                :, :, HD : HD + 1
            ]
            if ATTN_DT == F32:
                nc.vector.memset(ones_v, 1.0)
            else:
                nc.vector.tensor_copy(
                    _mm(ones_v, ATTN_DT),
                    ones_c[:, 0:1, None].to_broadcast(tuple(ones_v.shape)),
                )
            st["ones_c"] = ones_c

            for _rep in range(REPEAT):
                _phases(nc, tc, dbg, st, views)

    nc.compile()
    return nc


def _phases(nc, tc, dbg, st, views):
    qT_sb, kT_sb, v_sb = st["qT_sb"], st["kT_sb"], st["v_sb"]
    xT_v, wo_v, msk, out = views["xT_v"], views["wo_v"], views["msk"], views["out"]

    # ---------------- Phase A: QKV projections (one x stream) ----------------
    with (
        tc.tile_pool(name="wpool", bufs=1) as wpool,
        tc.tile_pool(name="xpool", bufs=2) as xpool,
        tc.tile_pool(name="ppsum", bufs=4, space="PSUM") as ppsum,
        tc.tile_pool(name="vpsum", bufs=4, space="PSUM") as vpsum,
    ):
        wq_sb = wpool.tile([P, KC, DG], F32, name="wq_sb")
        wk_sb = wpool.tile([P, KC, DG], F32, name="wk_sb")
        wv_sb = wpool.tile([P, KC, DG], F32, name="wv_sb")
        nc.sync.dma_start(_mm(wq_sb, PROJ_DT), _mm(views["wq_v"], PROJ_DT))
        nc.sync.dma_start(_mm(wk_sb, PROJ_DT), _mm(views["wk_v"], PROJ_DT))
        nc.sync.dma_start(_mm(wv_sb, PROJ_DT), _mm(views["wv_v"], PROJ_DT))
        for s in range(T // 512):
            x_sb = xpool.tile([P, KC, 512], F32, tag="x", name=f"x{s}")
            nc.sync.dma_start(
                _mm(x_sb, PROJ_DT), _mm(xT_v[:, :, 512 * s : 512 * (s + 1)], PROJ_DT)
            )
            for w_sb, b_key, dst, scale in (
                (wq_sb, "bq_sb", qT_sb, HD ** -0.5),
                (wk_sb, "bk_sb", kT_sb, 1.0),
            ):
                for d4 in range(DT4):
                    ps = ppsum.tile([P, 512], F32, tag="pp", name=f"pp{s}_{d4}")
                    for kc in range(KC):
                        nc.tensor.matmul(
                            ps,
                            _mm(w_sb[:, kc, P * d4 : P * (d4 + 1)], PROJ_DT),
                            _mm(x_sb[:, kc, :], PROJ_DT),
                            start=(kc == 0),
                            stop=(kc == KC - 1),
                        )
                    nc.scalar.activation(
                        _mm(dst[:, d4, 512 * s : 512 * (s + 1)], ATTN_DT),
                        ps,
                        Act.Identity,
                        bias=st[b_key][:, d4 : d4 + 1],
                        scale=scale,
                    )
            for t2 in range(4):
                tt = 4 * s + t2
                ps = vpsum.tile([P, DG], F32, tag="ppv", name=f"ppv{tt}")
                for kc in range(KC):
                    nc.tensor.matmul(
                        ps,
                        _mm(x_sb[:, kc, P * t2 : P * (t2 + 1)], PROJ_DT),
                        _mm(wv_sb[:, kc, :], PROJ_DT),
                        start=(kc == 0),
                        stop=(kc == KC - 1),
                    )
                nc.vector.tensor_copy(
                    _mm(
                        v_sb[:, tt].rearrange("p (h c) -> p h c", c=HD + 1)[:, :, :HD],
                        ATTN_DT,
                    ),
                    ps.rearrange("p (h c) -> p h c", c=HD),
                )

    # ---------------- Phase B: sparse attention ----------------
    with (
        tc.tile_pool(name="apool", bufs=1) as apool,
        tc.tile_pool(name="atpool", bufs=6) as atpool,
        tc.tile_pool(name="tmppool", bufs=2) as tmppool,
    ):
        ctxT_sb = apool.tile([P, DT4, T], F32, name="ctxT_sb")
        wo_sb = apool.tile([P, DT4, D], F32, name="wo_sb")
        nc.sync.dma_start(_mm(wo_sb, OPROJ_DT), _mm(wo_v, OPROJ_DT))
        m_strict = apool.tile([P, P], F32, name="m_strict")
        m_incl = apool.tile([P, P], F32, name="m_incl")
        m_diag = apool.tile([P, P], F32, name="m_diag")
        nc.sync.dma_start(m_strict, msk[0])
        nc.sync.dma_start(m_incl, msk[1])
        nc.sync.dma_start(m_diag, msk[2])
        ones_t = apool.tile([P, HD], F32, name="ones_t")  # row 64: K=1 bcast lhsT
        if BCAST_DT == F32:
            nc.vector.memset(ones_t, 1.0)
        else:
            nc.vector.tensor_copy(
                _mm(ones_t, BCAST_DT),
                st["ones_c"][:, 0:1].to_broadcast((P, HD)),
            )

        from contextlib import ExitStack as _ES

        _es = _ES()
        spsum = _es.enter_context(tc.tile_pool(name="spsum", bufs=4, space="PSUM"))
        cpsum = _es.enter_context(tc.tile_pool(name="cpsum", bufs=2, space="PSUM"))
        for h in range(HG):
            c, p0 = h // 2, HD * (h % 2)
            qh = qT_sb[p0 : p0 + HD, c, :]   # [64, 2048]
            kh = kT_sb[p0 : p0 + HD, c, :]
            for half in range(2):
                ctx = cpsum.tile([HD + 1, L], F32, tag="ctx", name=f"ctx{h}_{half}")
                mask = m_strict if half == 0 else m_incl
                for j in range(NT):
                    kv = kh[:, L + P * j : L + P * (j + 1)]                  # [64, 128]
                    vj = v_sb[:, NT + j, (HD + 1) * h : (HD + 1) * (h + 1)]  # [128, 65]
                    for a0, a1 in _chunks512(P * j, L):
                        n = a1 - a0
                        sc = spsum.tile(
                            [P, 512], F32, tag="sc", name=f"sc{h}_{j}_{half}_{a0}"
                        )[:, :n]
                        nc.tensor.matmul(
                            sc,
                            _mm(kv, ATTN_DT),
                            _mm(qh[:, L * half + a0 : L * half + a1], ATTN_DT),
                            start=True,
                            stop=True,
                        )
                        at = atpool.tile(
                            [P, 512], F32, tag="at", name=f"at{h}_{j}_{half}_{a0}"
                        )[:, :n]
                        nc.scalar.activation(_mm(at, ATTN_DT), sc, Act.Exp)
                        if a0 == P * j:
                            nc.vector.tensor_mul(
                                _mm(at[:, :P], ATTN_DT), at[:, :P], mask
                            )
                        if DBG and h == 0 and j == 0 and half == 1:
                            nc.sync.dma_start(dbg["dbg_at"][:, a0:a1], at)
                        # x0 half: stop on the last j touching this bank
                        last = half == 1 and (
                            (a1 <= 512 and j == 3) or (a0 >= 512 and j == NT - 1)
                        )
                        nc.tensor.matmul(
                            ctx[:, a0:a1],
                            _mm(vj, ATTN_DT),
                            _mm(at, ATTN_DT),
                            start=(j == 0),
                            stop=last,
                        )
                if half == 0:
                    # xt-xt block-diagonal tiles
                    for i in range(NT):
                        scd = spsum.tile(
                            [P, 512], F32, tag="sc", name=f"scd{h}_{i}"
                        )[:, :P]
                        nc.tensor.matmul(
                            scd,
                            _mm(kh[:, P * i : P * (i + 1)], ATTN_DT),
                            _mm(qh[:, P * i : P * (i + 1)], ATTN_DT),
                            start=True,
                            stop=True,
                        )
                        atd = atpool.tile(
                            [P, 512], F32, tag="at", name=f"atd{h}_{i}"
                        )[:, :P]
                        nc.scalar.activation(_mm(atd, ATTN_DT), scd, Act.Exp)
                        nc.vector.tensor_mul(_mm(atd, ATTN_DT), atd, m_diag)
                        nc.tensor.matmul(
                            ctx[:, P * i : P * (i + 1)],
                            _mm(v_sb[:, i, (HD + 1) * h : (HD + 1) * (h + 1)], ATTN_DT),
                            _mm(atd, ATTN_DT),
                            start=False,
                            stop=(i == 3 or i == NT - 1),
                        )
                if DBG:
                    ndc = tmppool.tile(
                        [HD + 1, L], F32, tag="ndc", name=f"ndc{h}_{half}"
                    )
                    nc.scalar.activation(ndc, ctx, Act.Copy)
                    nc.sync.dma_start(dbg["dbg_nd"][2 * h + half], ndc)
                # normalize: ctxT = ctx[:64] * (1 / denom), denom = row 64
                recip = tmppool.tile([P, L], F32, tag="recip", name=f"rc{h}_{half}")
                with nc.allow_low_precision(reason="deliberate f32r rounding"):
                    nc.vector.reciprocal(
                        _mm(recip[HD : HD + 1, :], BCAST_DT),
                        ctx[HD : HD + 1, :],
                    )
                rb = tmppool.tile([HD, L], F32, tag="rb", name=f"rb{h}_{half}")
                # PE broadcast: ones[1,64].T @ recip[1,n] -> [64, n]
                for c0 in range(0, L, 512):
                    bc = spsum.tile(
                        [P, 512], F32, tag="sc", name=f"bc{h}_{half}_{c0}"
                    )[:HD, :]
                    nc.tensor.matmul(
                        bc,
                        _mm(ones_t[HD : HD + 1, :], BCAST_DT),
                        _mm(recip[HD : HD + 1, c0 : c0 + 512], BCAST_DT),
                        start=True,
                        stop=True,
                    )
                    nc.vector.tensor_copy(rb[:, c0 : c0 + 512], bc)
                if h % 2 == 0:
                    nc.vector.tensor_mul(
                        _mm(ctxT_sb[:HD, c, L * half : L * (half + 1)], OPROJ_DT),
                        ctx[:HD, :],
                        rb,
                    )
                else:
                    cs = tmppool.tile([HD, L], F32, tag="cs", name=f"cs{h}_{half}")
                    nc.vector.tensor_mul(_mm(cs, OPROJ_DT), ctx[:HD, :], rb)
                    nc.sync.dma_start(
                        _mm(ctxT_sb[HD : 2 * HD, c, L * half : L * (half + 1)], OPROJ_DT),
                        _mm(cs, OPROJ_DT),
                    )

        if DBG:
            nc.sync.dma_start(dbg["dbg_qT"], qT_sb)
            nc.sync.dma_start(dbg["dbg_kT"], kT_sb)
            nc.sync.dma_start(dbg["dbg_v"], v_sb)
            nc.sync.dma_start(dbg["dbg_ctxT"], ctxT_sb)

        _es.close()

        # ---------------- Phase C: O-projection ----------------
        with tc.tile_pool(name="opsum", bufs=6, space="PSUM") as opsum:
            for tt in range(T // P):
                for nk in range(2):
                    ops = opsum.tile([P, 512], F32, tag="op", name=f"op{tt}_{nk}")
                    for cc in range(DT4):
                        nc.tensor.matmul(
                            ops,
                            _mm(ctxT_sb[:, cc, P * tt : P * (tt + 1)], OPROJ_DT),
                            _mm(wo_sb[:, cc, 512 * nk : 512 * (nk + 1)], OPROJ_DT),
                            start=(cc == 0),
                            stop=(cc == DT4 - 1),
                        )
                    osb = tmppool.tile([P, 512], F32, tag="osb", name=f"osb{tt}_{nk}")
                    nc.vector.tensor_copy(osb, ops)
                    nc.sync.dma_start(
                        out[P * tt : P * (tt + 1), 512 * nk : 512 * (nk + 1)], osb
                    )


def _masks():
    q = np.arange(P)[None, :] // BS
    k = np.arange(P)[:, None] // BS
    m = np.zeros((3, P, P), np.float32)
    m[0] = (q > k).astype(np.float32)    # strict (xt q vs x0 k, same tile)
    m[1] = (q >= k).astype(np.float32)   # incl (x0 q vs x0 k, same tile)
    m[2] = (q == k).astype(np.float32)   # diag (xt q vs xt k, same tile)
    return m


def kernel(x, Wq, bq, Wk, bk, Wv, bv, Wo, bo, block_size=4, **_):
    x = np.asarray(x, np.float32)
    Wq, bq = np.asarray(Wq, np.float32), np.asarray(bq, np.float32)
    Wk, bk = np.asarray(Wk, np.float32), np.asarray(bk, np.float32)
    Wv, bv = np.asarray(Wv, np.float32), np.asarray(bv, np.float32)
    Wo, bo = np.asarray(Wo, np.float32), np.asarray(bo, np.float32)

    if "nc" not in _CACHE:
        _CACHE["nc"] = _build()
    nc = _CACHE["nc"]

    masks = _masks()
    scale = HD ** -0.5
    in_maps = []
    for core in range(8):
        b, g = core // 2, core % 2
        cols = slice(DG * g, DG * (g + 1))
        in_maps.append(
            {
                "xT": np.ascontiguousarray(x[b].T),
                "wq": np.ascontiguousarray(Wq[:, cols]),
                "wk": np.ascontiguousarray(Wk[:, cols]),
                "wv": np.ascontiguousarray(Wv[:, cols]),
                "wo": np.ascontiguousarray(Wo[cols, :]),
                "bqs": np.ascontiguousarray(bq[cols]) * np.float32(scale),
                "bks": np.ascontiguousarray(bk[cols]),
                "msk": masks,
            }
        )

    _CACHE["last_in_maps"] = in_maps
    last_err = None
    for _attempt in range(6):
        try:
            res = run_bass_kernel_spmd(nc, in_maps, core_ids=list(range(8)), trace=False)
            break
        except Exception as e:  # transient NRT device flakes
            last_err = e
            msg = str(e)
            if "UNRECOVERABLE" not in msg and "UNAVAILABLE" not in msg:
                raise
            import time as _time

            import jax as _jax

            _time.sleep(5 * (_attempt + 1))
            try:
                _jax.clear_backends()
            except Exception:
                pass
    else:
        raise last_err

    corr = (bv @ Wo + bo).astype(np.float32)  # softmax rows sum to 1
    out = np.empty((B, T, D), np.float32)
    for b in range(B):
        out[b] = res.results[2 * b]["out"] + res.results[2 * b + 1]["out"] + corr
    return out


if __name__ == "__main__":
    rng = np.random.default_rng(0)
    inputs = {
        "x": rng.standard_normal((B, T, D)).astype(np.float32),
        "Wq": (rng.standard_normal((D, D)) / 32).astype(np.float32),
        "bq": np.zeros(D, np.float32),
        "Wk": (rng.standard_normal((D, D)) / 32).astype(np.float32),
        "bk": np.zeros(D, np.float32),
        "Wv": (rng.standard_normal((D, D)) / 32).astype(np.float32),
        "bv": np.zeros(D, np.float32),
        "Wo": (rng.standard_normal((D, D)) / 32).astype(np.float32),
        "bo": np.zeros(D, np.float32),
    }
    o = kernel(**inputs)
    print("ran", o.shape, o.dtype, float(np.abs(o).max()))
